# revision 5
# baseline (speedup 1.0000x reference)
"""LogitsProcessorWithLoRA on 8 trn2 NeuronCores.

Strategy (vocab tensor-parallel, uniform SPMD program on all 8 cores):
  - embedding [32256, 4096] is sharded over the ORG_VOCAB range (32000 cols,
    4000 per core).  Columns 32000:32256 of the base matmul are dead in the
    reference (overwritten by the lora-extra-vocab logits), so they are never
    computed.
  - core k additionally computes U_k = H @ embeddings_tensors[k].T (256 cols,
    unmasked); the host performs the per-token row-select (vLLM's
    sampler_indices gather) when stitching the full output.
  - the low-rank update delta = (mask * (H @ A^T)) @ B^T is folded into the
    same PSUM accumulation as the base matmul by augmenting the contraction
    dim: k-tiles 0..31 are the 4096 hidden dims, k-tile 32 is the 128 = 8
    loras x 16 rank dim.  xa^T (the [128, 1024] masked low-rank activations)
    is computed on-device first and becomes k-tile 32 of the stationary
    operand.
  - per-core output [1024, 4512] = [4000 base+delta | 256 U_k | 256 xdelta]
    where xdelta is the delta for the extra-vocab columns (identical on all
    cores; host uses core 0's).

All matmuls run in bf16 with fp32 PSUM accumulation.  All resident constants
(H^T, A^T, mask, B^T) ship as ONE packed bf16 input so they occupy a single
DMA completion lane — the first DVE instruction otherwise needs one sync wait
per lane and overflows the TensorTensor wait-command limit.
"""

import sys

import numpy as np

sys.path.insert(0, "/opt/trn_rl_repo")

import ml_dtypes

BF16 = ml_dtypes.bfloat16

# problem shapes (hardcoded per contract)
T = 1024
HID = 4096
L = 8
R = 16
ORG_VOCAB = 32000
EXTRA = 256
VOCAB = ORG_VOCAB + EXTRA
NCORES = 8
BASE = ORG_VOCAB // NCORES          # 4000 base vocab cols per core
LOCAL = BASE + EXTRA + EXTRA        # 4512 output cols per core
KO_H = HID // 128                   # 32 h k-tiles
LR = L * R                          # 128, the low-rank contraction dim
M_TILES = T // 128                  # 8 token partition tiles

# packed-constant column offsets (bf16 elements per partition)
OFF_HT = 0
OFF_AT = OFF_HT + KO_H * T          # 32768
OFF_MK = OFF_AT + KO_H * LR         # 36864
OFF_BT = OFF_MK + T                 # 37888
CST_COLS = OFF_BT + LOCAL           # 42400

_cache = {}


def _build_nc():
    import concourse.mybir as mybir
    import concourse.tile as tile
    from concourse import bacc
    from concourse.bass import ts

    nc = bacc.Bacc("TRN2", target_bir_lowering=False, debug=False)
    f32 = mybir.dt.float32
    bf16 = mybir.dt.bfloat16

    cst_d = nc.dram_tensor("cst", [128, CST_COLS], bf16, kind="ExternalInput")
    w1_d = nc.dram_tensor("w1", [128, KO_H, BASE + EXTRA], bf16, kind="ExternalInput")
    out_d = nc.dram_tensor("out", [128, M_TILES, LOCAL], f32, kind="ExternalOutput")

    with tile.TileContext(nc) as tc:
        with (
            tc.tile_pool(name="const", bufs=1) as const,
            tc.tile_pool(name="wpool", bufs=2) as wpool,
            tc.tile_pool(name="opool", bufs=4) as opool,
            tc.tile_pool(name="psum", bufs=8, space="PSUM") as psum,
        ):
            # resident constants: one tile, one DMA, one completion lane
            cst_sb = const.tile([128, CST_COLS], bf16)
            nc.sync.dma_start(cst_sb[:], cst_d[:])
            ht_sb = cst_sb[:, OFF_HT : OFF_HT + KO_H * T].rearrange(
                "p (ko t) -> p ko t", ko=KO_H
            )
            at_sb = cst_sb[:, OFF_AT : OFF_AT + KO_H * LR].rearrange(
                "p (ko m) -> p ko m", ko=KO_H
            )
            mk_sb = cst_sb[:, OFF_MK : OFF_MK + T]
            bt_sb = cst_sb[:, OFF_BT : OFF_BT + LOCAL]

            # xa^T (k-tile 32 of the augmented stationary operand)
            xat_sb = const.tile([128, T], bf16)

            # phase A: xa^T[lr, t] = sum_h A_all^T[h, lr] * H^T[h, t], masked
            for tp in range(T // 512):
                ps = psum.tile([128, 512], f32, tag="ps", name="ps")
                for ko in range(KO_H):
                    nc.tensor.matmul(
                        ps,
                        at_sb[:, ko, :],
                        ht_sb[:, ko, ts(tp, 512)],
                        start=(ko == 0),
                        stop=(ko == KO_H - 1),
                    )
                # apply per-token lora mask, cast to bf16
                nc.vector.tensor_tensor(
                    xat_sb[:, ts(tp, 512)],
                    ps,
                    mk_sb[:, ts(tp, 512)],
                    mybir.AluOpType.mult,
                )

            # phase B: out[t, c] accumulation over 33 k-tiles ---------------
            # chunks: (col0, width, has_h, has_lr)
            chunks = [(512 * i, 512, True, True) for i in range(8)]
            chunks.append((4096, 160, True, False))   # tail of U_k region
            chunks.append((4256, 256, False, True))   # xdelta (k-tile 32 only)

            for c0, w, has_h, has_lr in chunks:
                if has_h:
                    wt = wpool.tile([128, KO_H, 512], bf16, tag="w", name="wt")
                    nc.sync.dma_start(wt[:, :, :w], w1_d[:, :, c0 : c0 + w])
                for m in range(M_TILES):
                    ps = psum.tile([128, 512], f32, tag="ps", name="ps")[:, :w]
                    n_mm = (KO_H if has_h else 0) + (1 if has_lr else 0)
                    i = 0
                    if has_h:
                        for ko in range(KO_H):
                            nc.tensor.matmul(
                                ps,
                                ht_sb[:, ko, ts(m, 128)],
                                wt[:, ko, :w],
                                start=(i == 0),
                                stop=(i == n_mm - 1),
                            )
                            i += 1
                    if has_lr:
                        nc.tensor.matmul(
                            ps,
                            xat_sb[:, ts(m, 128)],
                            bt_sb[:, c0 : c0 + w],
                            start=(i == 0),
                            stop=(i == n_mm - 1),
                        )
                    ot = opool.tile([128, 512], f32, tag="o", name="ot")[:, :w]
                    nc.vector.tensor_copy(ot, ps)
                    nc.sync.dma_start(out_d[:, m, c0 : c0 + w], ot)

    nc.compile()
    return nc


def _get_nc():
    if "nc" not in _cache:
        _cache["nc"] = _build_nc()
    return _cache["nc"]


def _prep_inputs(hidden_states, embedding, embeddings_tensors, lora_a_stacked,
                 lora_b_stacked, indices):
    """Host-side shard + layout prep (weight-layout transform, as a TP
    inference server does at load time)."""
    hs = np.asarray(hidden_states, np.float32)
    emb = np.asarray(embedding, np.float32)
    et = np.asarray(embeddings_tensors, np.float32)
    la = np.asarray(lora_a_stacked, np.float32)
    lb = np.asarray(lora_b_stacked, np.float32)
    idx = np.asarray(indices, np.int32)

    cst = np.empty((128, CST_COLS), BF16)
    # ht[p, ko*T + t] = H[t, ko*128+p]
    cst[:, OFF_HT : OFF_HT + KO_H * T] = (
        hs.T.reshape(KO_H, 128, T).transpose(1, 0, 2).reshape(128, KO_H * T)
    ).astype(BF16)
    # at[p, ko*LR + m] = A_all[m, ko*128+p];  A_all[l*16+r, h] = lora_a[l, r, h]
    a_all = la.reshape(LR, HID)
    cst[:, OFF_AT : OFF_AT + KO_H * LR] = (
        a_all.T.reshape(KO_H, 128, LR).transpose(1, 0, 2).reshape(128, KO_H * LR)
    ).astype(BF16)
    # mask[p, t] = 1.0 if indices[t] == p//16 else 0  (exact in bf16)
    lr_l = np.arange(LR) // R
    cst[:, OFF_MK : OFF_MK + T] = (idx[None, :] == lr_l[:, None]).astype(BF16)

    in_maps = []
    for k in range(NCORES):
        c = cst.copy()
        bt = np.zeros((LR, LOCAL), np.float32)
        bk = lb[:, BASE * k : BASE * (k + 1), :]          # [L, 4000, R]
        bt[:, :BASE] = bk.transpose(0, 2, 1).reshape(LR, BASE)
        bx = lb[:, ORG_VOCAB:VOCAB, :]                    # [L, 256, R]
        bt[:, BASE + EXTRA :] = bx.transpose(0, 2, 1).reshape(LR, EXTRA)
        c[:, OFF_BT:] = bt.astype(BF16)

        w = np.concatenate([emb[BASE * k : BASE * (k + 1)], et[k]], axis=0)
        # w1[p, ko, n] = W[n, ko*128+p]
        w1 = np.ascontiguousarray(
            w.T.reshape(KO_H, 128, BASE + EXTRA).transpose(1, 0, 2)
        ).astype(BF16)
        in_maps.append({"cst": c, "w1": w1})
    return in_maps, idx


def _combine(results, idx):
    """Stitch per-core [128, 8, 4512] outputs into the full [1024, 32256]."""
    local = [
        np.asarray(r["out"]).transpose(1, 0, 2).reshape(T, LOCAL) for r in results
    ]
    final = np.empty((T, VOCAB), np.float32)
    for k in range(NCORES):
        final[:, BASE * k : BASE * (k + 1)] = local[k][:, :BASE]
    u = np.stack([lc[:, BASE : BASE + EXTRA] for lc in local], axis=0)  # [8,T,256]
    xdelta = local[0][:, BASE + EXTRA :]
    tpos = np.arange(T)
    safe = np.where(idx >= 0, idx, 0)
    sel = u[safe, tpos, :]
    extra = np.where((idx >= 0)[:, None], sel + xdelta, -np.inf)
    final[:, ORG_VOCAB:] = extra
    return final


def _run(inputs, trace=False):
    from concourse.bass_utils import run_bass_kernel_spmd

    in_maps, idx = _prep_inputs(**inputs)
    nc = _get_nc()
    res = run_bass_kernel_spmd(nc, in_maps, core_ids=list(range(NCORES)),
                               trace=trace)
    return _combine(res.results, idx), res


def kernel(**inputs) -> np.ndarray:
    out, _ = _run(inputs, trace=False)
    return out
